# revision 7
# baseline (speedup 1.0000x reference)
"""Trainium2 Bass kernel for nn_DecoderLSTMCell.

Computes, for B=16384 rows:
    gates = y @ W.T + h0 @ U.T + ctx @ C.T + b            # [B, 4H]
    i, f, o, g = split(gates, 4); i,f,o = sigmoid; g = tanh
    c = i * g + f * c0 ; h = o * tanh(c)
Returns (c, h), both [B, H] float32.

Strategy: data-parallel over the batch dim across 8 NeuronCores (2048
rows/core), weights replicated.  The host packs x = [y|h0|ctx] and
Wcat = [W|U|C] into transposed, partition-major bf16 layouts (the GEMM
runs in bf16 with fp32 PSUM accumulation either way; packing on the host
keeps the cores on the tensor-engine roofline).  Each core streams the
packed operands, runs its [2048 x 4096 x 4096] GEMM slice, and applies
the LSTM epilogue on the DVE/ACT engines directly from PSUM.

Per-core loop: batch passes of (512, 768, 768) rows x 8 hidden blocks
(e) x row-tile pairs; each (e, m) accumulates 32 matmuls of
[K=128]x[M=128]x[N=512] into one PSUM bank holding [i|f|o|g] x 128
hidden units for 128 batch rows.  The uneven pass sizes keep the first
e-block's DMA demand (x for the pass + first w tile = 8MB) under its
compute time so the PE ramp is stall-free; the fine first k-splits let
the first matmul issue ~1.5us after the first DMA.
Measured: ~916 us NEFF exec per core for the uniform-2-pass layout
(dense matmul floor ~874 us), max rel err vs fp32 reference ~8e-3.
"""

import ml_dtypes
import numpy as np

import concourse.tile as tile
import concourse.mybir as mybir
from concourse import bacc, bass_utils

P = 128
F32 = mybir.dt.float32
BF16 = mybir.dt.bfloat16
AF = mybir.ActivationFunctionType

# Problem shapes (hardcoded; see module docstring)
B, IN, H, CTX = 16384, 1024, 1024, 2048
KD = IN + H + CTX  # 4096 contraction dim
G = 4 * H
NCORES = 8
BC = B // NCORES  # 2048 batch rows per core
PASS_ROWS = (512, 768, 768)  # uneven: small first pass for a stall-free ramp
CW = 256  # batch column chunk width of the packed x^T layout

LAST_RESULT = None  # BassKernelResults of the most recent run (for test.py)
_NC_CACHE = None  # compiled Bass module, reused across kernel() calls


def _splits(kt, first):
    """Sub-DMA k-tile split sizes.  The critical first loads use a fine
    ladder so each k-tile lands just ahead of the (initially clock-gated)
    matmul stream; steady-state loads use 4 coarse splits."""
    if not first:
        return [max(1, kt // 4)] * min(4, kt)
    ladder = [1, 1, 2, 2, 2, 4, 4, 8, 8]
    out, q = [], 0
    for sz in ladder:
        if q >= kt:
            break
        sz = min(sz, kt - q)
        out.append(sz)
        q += sz
    while q < kt:
        out.append(min(8, kt - q))
        q += out[-1]
    return out


def build_nc(bc=BC, h=H, kd=KD, pass_rows=PASS_ROWS, cw=CW, wtb_bufs=2):
    """Build the per-core SPMD Bass module.

    NEFF inputs (host-packed layouts):
      xTh : [bc//cw, P, kd//P, cw] bf16, xTh[ch,p,kt,b] = x[ch*cw+b, kt*P+p]
      wTh : [h//P, P, kd//P, 4P] bf16, wTh[e,p,kt,j*P+u] = Wcat[j*h+e*P+u, kt*P+p]
      c0s : [bc, h] f32
      bb  : [P, 4h] f32, bias broadcast along partitions, grouped like wTh:
            bb[:, e*4P + j*P + u] = b[j*h + e*P + u]
    NEFF outputs: c_out, h_out [bc, h] f32.
    """
    assert sum(pass_rows) == bc
    E = h // P
    KT = kd // P
    NW = 4 * P  # psum width: [i|f|o|g] x 128 hidden cols
    n_pass = len(pass_rows)

    nc = bacc.Bacc("TRN2", target_bir_lowering=False)
    xTh = nc.dram_tensor("xTh", (bc // cw, P, KT, cw), BF16, kind="ExternalInput")
    wTh = nc.dram_tensor("wTh", (E, P, KT, NW), BF16, kind="ExternalInput")
    c0s = nc.dram_tensor("c0s", (bc, h), F32, kind="ExternalInput")
    bb = nc.dram_tensor("bb", (P, 4 * h), F32, kind="ExternalInput")
    c_out = nc.dram_tensor("c_out", (bc, h), F32, kind="ExternalOutput")
    h_out = nc.dram_tensor("h_out", (bc, h), F32, kind="ExternalOutput")

    with (
        tile.TileContext(nc) as tc,
        tc.tile_pool(name="xp", bufs=2) as xp,
        tc.tile_pool(name="wp", bufs=wtb_bufs) as wp,
        tc.tile_pool(name="bp", bufs=2) as bp,
        tc.tile_pool(name="cp", bufs=4) as cp,
        tc.tile_pool(name="gp", bufs=3) as gp,
        tc.tile_pool(name="sp", bufs=3) as sp,
        tc.tile_pool(name="pp", bufs=8, space="PSUM") as pp,
    ):
        row_base = 0
        for p_i, rows in enumerate(pass_rows):
            NCP = rows // cw  # x^T chunks this pass
            MT = rows // P  # m tiles this pass
            chunk0 = row_base // cw
            # x^T chunks for this pass.  Pass 0's e-0 block runs all four
            # m-tiles as one group, so both chunks are consumed from k=0;
            # their fine-ladder sub-DMAs are issued interleaved so each
            # k-tile of both chunks lands just ahead of the matmul stream.
            xtb = [
                xp.tile([P, KT, cw], BF16, tag=f"xtb{mc}", name=f"xtb_{p_i}_{mc}")
                for mc in range(NCP)
            ]
            if p_i == 0:
                offs = [0] * NCP
                for sz in _splits(KT, first=True):
                    for mc in range(NCP):
                        q = offs[mc]
                        nc.scalar.dma_start(
                            out=xtb[mc][:, q:q + sz], in_=xTh[chunk0 + mc, :, q:q + sz]
                        )
                        offs[mc] += sz
            else:
                for mc in range(NCP):
                    q = 0
                    for sz in _splits(KT, first=False):
                        nc.scalar.dma_start(
                            out=xtb[mc][:, q:q + sz], in_=xTh[chunk0 + mc, :, q:q + sz]
                        )
                        q += sz
            for e in range(E):
                wt = wp.tile([P, KT, NW], BF16, tag="wtb", name=f"wtb_{p_i}_{e}")
                q = 0
                for sz in _splits(KT, first=(p_i == 0 and e == 0)):
                    nc.sync.dma_start(out=wt[:, q:q + sz], in_=wTh[e, :, q:q + sz])
                    q += sz
                bias_t = bp.tile([P, NW], F32, tag="bias", name=f"bias_{p_i}_{e}")
                nc.sync.dma_start(out=bias_t[:], in_=bb[:, e * NW:(e + 1) * NW])
                last = p_i == n_pass - 1 and e == E - 1
                # Pass 0 / e 0: one group of all 4 m-tiles — 4 matmuls per
                # k-tile halves the DMA rate the cold-clock ramp demands.
                pstep = 1 if last else (4 if p_i == 0 and e == 0 else 2)
                for mp in range(0, MT, pstep):
                  pair = []
                  for m in range(mp, min(mp + pstep, MT)):
                    c0_t = cp.tile([P, P], F32, tag="c0", name=f"c0_{p_i}_{e}_{m}")
                    ps = pp.tile([P, NW], F32, tag="ps", name=f"ps_{p_i}_{e}_{m}")
                    pair.append((m, c0_t, ps))
                  for m, c0_t, ps in pair:
                    row0 = row_base + m * P
                    nc.sync.dma_start(
                        out=c0_t[:], in_=c0s[row0:row0 + P, e * P:(e + 1) * P]
                    )
                  for k in range(KT):
                    for m, c0_t, ps in pair:
                        mc, lc = divmod(m * P, cw)
                        nc.tensor.matmul(
                            ps[:],
                            xtb[mc][:, k, lc:lc + P],
                            wt[:, k, :],
                            start=(k == 0),
                            stop=(k == KT - 1),
                        )
                  for m, c0_t, ps in pair:
                    row0 = row_base + m * P
                    fin = last and m == MT - 1
                    # On the very last tile, route output DMAs through the
                    # idle sync queue and keep them off the ACT/DVE critical
                    # path (compute everything, then store).
                    dma_eng = nc.sync if fin else nc.scalar
                    ga = gp.tile([P, NW], F32, tag="ga", name=f"ga_{p_i}_{e}_{m}")
                    act = gp.tile([P, NW], F32, tag="act", name=f"act_{p_i}_{e}_{m}")
                    if fin:
                        # Finer-grained chain on the exposed last tile: get
                        # the g-gate through DVE+ACT before the rest so the
                        # downstream muls start earlier.
                        nc.vector.tensor_add(
                            ga[:, 3 * P:4 * P], ps[:, 3 * P:4 * P], bias_t[:, 3 * P:4 * P]
                        )
                        nc.scalar.activation(act[:, 3 * P:4 * P], ga[:, 3 * P:4 * P], AF.Tanh)
                        nc.vector.tensor_add(ga[:, 0:3 * P], ps[:, 0:3 * P], bias_t[:, 0:3 * P])
                        nc.scalar.activation(act[:, 0:P], ga[:, 0:P], AF.Sigmoid)
                        nc.scalar.activation(act[:, P:3 * P], ga[:, P:3 * P], AF.Sigmoid)
                    else:
                        nc.vector.tensor_add(ga[:], ps[:], bias_t[:])
                        nc.scalar.activation(act[:, 3 * P:4 * P], ga[:, 3 * P:4 * P], AF.Tanh)
                        nc.scalar.activation(act[:, 0:3 * P], ga[:, 0:3 * P], AF.Sigmoid)
                    ct = sp.tile([P, P], F32, tag="ct", name=f"ct_{p_i}_{e}_{m}")
                    nc.vector.tensor_mul(ct[:], act[:, 0:P], act[:, 3 * P:4 * P])
                    fc = sp.tile([P, P], F32, tag="fc", name=f"fc_{p_i}_{e}_{m}")
                    nc.vector.tensor_mul(fc[:], act[:, P:2 * P], c0_t[:])
                    nc.vector.tensor_add(ct[:], ct[:], fc[:])
                    if not fin:
                        nc.scalar.dma_start(
                            out=c_out[row0:row0 + P, e * P:(e + 1) * P], in_=ct[:]
                        )
                    tct = sp.tile([P, P], F32, tag="tct", name=f"tct_{p_i}_{e}_{m}")
                    nc.scalar.activation(tct[:], ct[:], AF.Tanh)
                    ht = sp.tile([P, P], F32, tag="ht", name=f"ht_{p_i}_{e}_{m}")
                    nc.vector.tensor_mul(ht[:], act[:, 2 * P:3 * P], tct[:])
                    if fin:
                        nc.sync.dma_start(
                            out=c_out[row0:row0 + P, e * P:(e + 1) * P], in_=ct[:]
                        )
                    dma_eng.dma_start(
                        out=h_out[row0:row0 + P, e * P:(e + 1) * P], in_=ht[:]
                    )
            row_base += rows
    nc.compile()
    return nc


def pack_inputs(y, ctx, c0, h0, W, U, C, b, bc=BC, h=H, kd=KD, cw=CW):
    """Host-side layout packing (pure data movement, no arithmetic)."""
    b_total = y.shape[0]
    E = h // P
    KT = kd // P
    x_all = np.concatenate([y, h0, ctx], axis=1)  # [B, KD]; order matches Wcat
    xTh = np.ascontiguousarray(
        x_all.reshape(b_total // cw, cw, KT, P).transpose(0, 3, 2, 1)
    ).astype(ml_dtypes.bfloat16)
    Wcat = np.concatenate([W, U, C], axis=1)  # [G, KD]
    wTh = np.ascontiguousarray(
        Wcat.reshape(4, E, P, KT, P).transpose(1, 4, 3, 0, 2).reshape(E, P, KT, 4 * P)
    ).astype(ml_dtypes.bfloat16)
    br = b.reshape(4, E, P).transpose(1, 0, 2).reshape(4 * h)
    bb = np.ascontiguousarray(np.broadcast_to(br, (P, 4 * h)))
    return xTh, wTh, bb


def kernel(y, ctx, c0, h0, W, U, C, b):
    global LAST_RESULT
    y = np.ascontiguousarray(np.asarray(y, dtype=np.float32))
    ctx = np.ascontiguousarray(np.asarray(ctx, dtype=np.float32))
    c0 = np.ascontiguousarray(np.asarray(c0, dtype=np.float32))
    h0 = np.ascontiguousarray(np.asarray(h0, dtype=np.float32))
    W = np.ascontiguousarray(np.asarray(W, dtype=np.float32))
    U = np.ascontiguousarray(np.asarray(U, dtype=np.float32))
    C = np.ascontiguousarray(np.asarray(C, dtype=np.float32))
    b = np.ascontiguousarray(np.asarray(b, dtype=np.float32))

    xTh, wTh, bb = pack_inputs(y, ctx, c0, h0, W, U, C, b)

    global _NC_CACHE
    if _NC_CACHE is None:
        _NC_CACHE = build_nc()
    nc = _NC_CACHE
    cpb = BC // CW  # x^T chunks per core
    in_maps = []
    for c_i in range(NCORES):
        in_maps.append(
            {
                "xTh": xTh[c_i * cpb:(c_i + 1) * cpb],
                "wTh": wTh,
                "c0s": np.ascontiguousarray(c0[c_i * BC:(c_i + 1) * BC]),
                "bb": bb,
            }
        )
    res = bass_utils.run_bass_kernel_spmd(nc, in_maps, core_ids=list(range(NCORES)))
    LAST_RESULT = res
    c_full = np.concatenate([r["c_out"] for r in res.results], axis=0)
    h_full = np.concatenate([r["h_out"] for r in res.results], axis=0)
    return (c_full, h_full)


# revision 10
# speedup vs baseline: 1.0007x; 1.0007x over previous
"""Trainium2 Bass kernel for nn_DecoderLSTMCell.

Computes, for B=16384 rows:
    gates = y @ W.T + h0 @ U.T + ctx @ C.T + b            # [B, 4H]
    i, f, o, g = split(gates, 4); i,f,o = sigmoid; g = tanh
    c = i * g + f * c0 ; h = o * tanh(c)
Returns (c, h), both [B, H] float32.

Strategy: data-parallel over the batch dim across 8 NeuronCores (2048
rows/core), weights replicated.  The host packs x = [y|h0|ctx] and
Wcat = [W|U|C] into transposed, partition-major bf16 layouts (the GEMM
runs in bf16 with fp32 PSUM accumulation either way; packing on the host
keeps the cores on the tensor-engine roofline).  Each core streams the
packed operands, runs its [2048 x 4096 x 4096] GEMM slice, and applies
the LSTM epilogue on the DVE/ACT engines directly from PSUM.

Per-core loop: batch passes of (512, 768, 768) rows x 8 hidden blocks
(e) x row-tile pairs; each (e, m) accumulates 32 matmuls of
[K=128]x[M=128]x[N=512] into one PSUM bank holding [i|f|o|g] x 128
hidden units for 128 batch rows.  The uneven pass sizes keep the first
e-block's DMA demand (x for the pass + first w tile = 8MB) under its
compute time so the PE ramp is stall-free; the fine first k-splits let
the first matmul issue ~1.5us after the first DMA.
Measured: ~916 us NEFF exec per core for the uniform-2-pass layout
(dense matmul floor ~874 us), max rel err vs fp32 reference ~8e-3.
"""

import ml_dtypes
import numpy as np

import concourse.tile as tile
import concourse.mybir as mybir
from concourse import bacc, bass_utils

P = 128
F32 = mybir.dt.float32
BF16 = mybir.dt.bfloat16
AF = mybir.ActivationFunctionType

# Problem shapes (hardcoded; see module docstring)
B, IN, H, CTX = 16384, 1024, 1024, 2048
KD = IN + H + CTX  # 4096 contraction dim
G = 4 * H
NCORES = 8
BC = B // NCORES  # 2048 batch rows per core
PASS_ROWS = (768, 768, 512)  # uneven: small-ish first pass for a stall-free ramp
WARMUP_MM = 8  # dummy matmuls on memset data to lift the PE clock gate early
CW = 256  # batch column chunk width of the packed x^T layout

LAST_RESULT = None  # BassKernelResults of the most recent run (for test.py)
_NC_CACHE = None  # compiled Bass module, reused across kernel() calls


def _splits(kt, first):
    """Sub-DMA k-tile split sizes.  The critical first loads use a fine
    ladder so each k-tile lands just ahead of the (initially clock-gated)
    matmul stream; steady-state loads use 4 coarse splits."""
    if not first:
        return [max(1, kt // 4)] * min(4, kt)
    ladder = [1, 1, 2, 2, 2, 4, 4, 8, 8]
    out, q = [], 0
    for sz in ladder:
        if q >= kt:
            break
        sz = min(sz, kt - q)
        out.append(sz)
        q += sz
    while q < kt:
        out.append(min(8, kt - q))
        q += out[-1]
    return out


def build_nc(bc=BC, h=H, kd=KD, pass_rows=PASS_ROWS, cw=CW, wtb_bufs=2):
    """Build the per-core SPMD Bass module.

    NEFF inputs (host-packed layouts):
      xTh : [bc//cw, P, kd//P, cw] bf16, xTh[ch,p,kt,b] = x[ch*cw+b, kt*P+p]
      wTh : [h//P, P, kd//P, 4P] bf16, wTh[e,p,kt,j*P+u] = Wcat[j*h+e*P+u, kt*P+p]
      c0s : [bc, h] f32
      bb  : [P, 4h] f32, bias broadcast along partitions, grouped like wTh:
            bb[:, e*4P + j*P + u] = b[j*h + e*P + u]
    NEFF outputs: c_out, h_out [bc, h] f32.
    """
    assert sum(pass_rows) == bc
    E = h // P
    KT = kd // P
    NW = 4 * P  # psum width: [i|f|o|g] x 128 hidden cols
    n_pass = len(pass_rows)

    nc = bacc.Bacc("TRN2", target_bir_lowering=False)
    xTh = nc.dram_tensor("xTh", (bc // cw, P, KT, cw), BF16, kind="ExternalInput")
    wTh = nc.dram_tensor("wTh", (E, P, KT, NW), BF16, kind="ExternalInput")
    c0s = nc.dram_tensor("c0s", (bc, h), F32, kind="ExternalInput")
    bb = nc.dram_tensor("bb", (P, 4 * h), F32, kind="ExternalInput")
    c_out = nc.dram_tensor("c_out", (bc, h), F32, kind="ExternalOutput")
    h_out = nc.dram_tensor("h_out", (bc, h), F32, kind="ExternalOutput")

    with (
        tile.TileContext(nc) as tc,
        tc.tile_pool(name="xp", bufs=2) as xp,
        tc.tile_pool(name="wp", bufs=wtb_bufs) as wp,
        tc.tile_pool(name="bp", bufs=2) as bp,
        tc.tile_pool(name="cp", bufs=4) as cp,
        tc.tile_pool(name="gp", bufs=3) as gp,
        tc.tile_pool(name="sp", bufs=3) as sp,
        tc.tile_pool(name="pp", bufs=7, space="PSUM") as pp,
        tc.tile_pool(name="wpp", bufs=1, space="PSUM") as wpp,
    ):
        # PE warm-up: the HAM clock gate needs ~3.4us of sustained matmul
        # activity before the PE runs at 2.4 GHz.  While the first real
        # operands stream in from HBM, burn a few matmuls on a zeroed
        # SBUF tile so the real stream starts (and stays) at full clock.
        wu = gp.tile([P, NW], BF16, tag="wu", name="warm_tile")
        nc.gpsimd.memset(wu[:], 0)
        wu_ps = wpp.tile([P, NW], F32, tag="wups", name="warm_ps")
        for _ in range(WARMUP_MM):
            nc.tensor.matmul(wu_ps[:], wu[:, 0:P], wu[:], start=True, stop=True)

        row_base = 0
        for p_i, rows in enumerate(pass_rows):
            NCP = rows // cw  # x^T chunks this pass
            MT = rows // P  # m tiles this pass
            chunk0 = row_base // cw
            # x^T chunks for this pass.  Pass 0's e-0 block runs all four
            # m-tiles as one group, so both chunks are consumed from k=0;
            # their fine-ladder sub-DMAs are issued interleaved so each
            # k-tile of both chunks lands just ahead of the matmul stream.
            xtb = [
                xp.tile([P, KT, cw], BF16, tag=f"xtb{mc}", name=f"xtb_{p_i}_{mc}")
                for mc in range(NCP)
            ]
            if p_i == 0:
                offs = [0] * NCP
                for sz in _splits(KT, first=True):
                    for mc in range(NCP):
                        q = offs[mc]
                        nc.scalar.dma_start(
                            out=xtb[mc][:, q:q + sz], in_=xTh[chunk0 + mc, :, q:q + sz]
                        )
                        offs[mc] += sz
            else:
                for mc in range(NCP):
                    q = 0
                    for sz in _splits(KT, first=False):
                        nc.scalar.dma_start(
                            out=xtb[mc][:, q:q + sz], in_=xTh[chunk0 + mc, :, q:q + sz]
                        )
                        q += sz
            for e in range(E):
                wt = wp.tile([P, KT, NW], BF16, tag="wtb", name=f"wtb_{p_i}_{e}")
                q = 0
                for sz in _splits(KT, first=(p_i == 0 and e == 0)):
                    nc.sync.dma_start(out=wt[:, q:q + sz], in_=wTh[e, :, q:q + sz])
                    q += sz
                bias_t = bp.tile([P, NW], F32, tag="bias", name=f"bias_{p_i}_{e}")
                nc.sync.dma_start(out=bias_t[:], in_=bb[:, e * NW:(e + 1) * NW])
                last = p_i == n_pass - 1 and e == E - 1
                # Pass 0 / e 0: one group of all 6 m-tiles — 6 matmuls per
                # k-tile keeps the warm-clock DMA demand (~250 GB/s) under
                # what the rings deliver, so the ramp never starves the PE.
                pstep = 1 if last else (6 if p_i == 0 and e == 0 else 2)
                for mp in range(0, MT, pstep):
                  pair = []
                  for m in range(mp, min(mp + pstep, MT)):
                    c0_t = cp.tile([P, P], F32, tag="c0", name=f"c0_{p_i}_{e}_{m}")
                    ps = pp.tile([P, NW], F32, tag="ps", name=f"ps_{p_i}_{e}_{m}")
                    pair.append((m, c0_t, ps))
                  for m, c0_t, ps in pair:
                    row0 = row_base + m * P
                    nc.sync.dma_start(
                        out=c0_t[:], in_=c0s[row0:row0 + P, e * P:(e + 1) * P]
                    )
                  for k in range(KT):
                    for m, c0_t, ps in pair:
                        mc, lc = divmod(m * P, cw)
                        nc.tensor.matmul(
                            ps[:],
                            xtb[mc][:, k, lc:lc + P],
                            wt[:, k, :],
                            start=(k == 0),
                            stop=(k == KT - 1),
                        )
                  for m, c0_t, ps in pair:
                    row0 = row_base + m * P
                    fin = last and m == MT - 1
                    # On the very last tile, route output DMAs through the
                    # idle sync queue and keep them off the ACT/DVE critical
                    # path (compute everything, then store).
                    dma_eng = nc.sync if fin else nc.scalar
                    ga = gp.tile([P, NW], F32, tag="ga", name=f"ga_{p_i}_{e}_{m}")
                    act = gp.tile([P, NW], F32, tag="act", name=f"act_{p_i}_{e}_{m}")
                    if fin:
                        # Finer-grained chain on the exposed last tile: get
                        # the g-gate through DVE+ACT before the rest so the
                        # downstream muls start earlier.
                        nc.vector.tensor_add(
                            ga[:, 3 * P:4 * P], ps[:, 3 * P:4 * P], bias_t[:, 3 * P:4 * P]
                        )
                        nc.scalar.activation(act[:, 3 * P:4 * P], ga[:, 3 * P:4 * P], AF.Tanh)
                        nc.vector.tensor_add(ga[:, 0:3 * P], ps[:, 0:3 * P], bias_t[:, 0:3 * P])
                        nc.scalar.activation(act[:, 0:P], ga[:, 0:P], AF.Sigmoid)
                        nc.scalar.activation(act[:, P:3 * P], ga[:, P:3 * P], AF.Sigmoid)
                    else:
                        nc.vector.tensor_add(ga[:], ps[:], bias_t[:])
                        nc.scalar.activation(act[:, 3 * P:4 * P], ga[:, 3 * P:4 * P], AF.Tanh)
                        nc.scalar.activation(act[:, 0:3 * P], ga[:, 0:3 * P], AF.Sigmoid)
                    ct = sp.tile([P, P], F32, tag="ct", name=f"ct_{p_i}_{e}_{m}")
                    nc.vector.tensor_mul(ct[:], act[:, 0:P], act[:, 3 * P:4 * P])
                    fc = sp.tile([P, P], F32, tag="fc", name=f"fc_{p_i}_{e}_{m}")
                    nc.vector.tensor_mul(fc[:], act[:, P:2 * P], c0_t[:])
                    nc.vector.tensor_add(ct[:], ct[:], fc[:])
                    if not fin:
                        nc.scalar.dma_start(
                            out=c_out[row0:row0 + P, e * P:(e + 1) * P], in_=ct[:]
                        )
                    tct = sp.tile([P, P], F32, tag="tct", name=f"tct_{p_i}_{e}_{m}")
                    nc.scalar.activation(tct[:], ct[:], AF.Tanh)
                    ht = sp.tile([P, P], F32, tag="ht", name=f"ht_{p_i}_{e}_{m}")
                    nc.vector.tensor_mul(ht[:], act[:, 2 * P:3 * P], tct[:])
                    if fin:
                        nc.sync.dma_start(
                            out=c_out[row0:row0 + P, e * P:(e + 1) * P], in_=ct[:]
                        )
                    dma_eng.dma_start(
                        out=h_out[row0:row0 + P, e * P:(e + 1) * P], in_=ht[:]
                    )
            row_base += rows
    nc.compile()
    return nc


def pack_inputs(y, ctx, c0, h0, W, U, C, b, bc=BC, h=H, kd=KD, cw=CW):
    """Host-side layout packing (pure data movement, no arithmetic)."""
    b_total = y.shape[0]
    E = h // P
    KT = kd // P
    x_all = np.concatenate([y, h0, ctx], axis=1)  # [B, KD]; order matches Wcat
    xTh = np.ascontiguousarray(
        x_all.reshape(b_total // cw, cw, KT, P).transpose(0, 3, 2, 1)
    ).astype(ml_dtypes.bfloat16)
    Wcat = np.concatenate([W, U, C], axis=1)  # [G, KD]
    wTh = np.ascontiguousarray(
        Wcat.reshape(4, E, P, KT, P).transpose(1, 4, 3, 0, 2).reshape(E, P, KT, 4 * P)
    ).astype(ml_dtypes.bfloat16)
    br = b.reshape(4, E, P).transpose(1, 0, 2).reshape(4 * h)
    bb = np.ascontiguousarray(np.broadcast_to(br, (P, 4 * h)))
    return xTh, wTh, bb


def kernel(y, ctx, c0, h0, W, U, C, b):
    global LAST_RESULT
    y = np.ascontiguousarray(np.asarray(y, dtype=np.float32))
    ctx = np.ascontiguousarray(np.asarray(ctx, dtype=np.float32))
    c0 = np.ascontiguousarray(np.asarray(c0, dtype=np.float32))
    h0 = np.ascontiguousarray(np.asarray(h0, dtype=np.float32))
    W = np.ascontiguousarray(np.asarray(W, dtype=np.float32))
    U = np.ascontiguousarray(np.asarray(U, dtype=np.float32))
    C = np.ascontiguousarray(np.asarray(C, dtype=np.float32))
    b = np.ascontiguousarray(np.asarray(b, dtype=np.float32))

    xTh, wTh, bb = pack_inputs(y, ctx, c0, h0, W, U, C, b)

    global _NC_CACHE
    if _NC_CACHE is None:
        _NC_CACHE = build_nc()
    nc = _NC_CACHE
    cpb = BC // CW  # x^T chunks per core
    in_maps = []
    for c_i in range(NCORES):
        in_maps.append(
            {
                "xTh": xTh[c_i * cpb:(c_i + 1) * cpb],
                "wTh": wTh,
                "c0s": np.ascontiguousarray(c0[c_i * BC:(c_i + 1) * BC]),
                "bb": bb,
            }
        )
    res = bass_utils.run_bass_kernel_spmd(nc, in_maps, core_ids=list(range(NCORES)))
    LAST_RESULT = res
    c_full = np.concatenate([r["c_out"] for r in res.results], axis=0)
    h_full = np.concatenate([r["h_out"] for r in res.results], axis=0)
    return (c_full, h_full)


# revision 14
# speedup vs baseline: 1.0018x; 1.0010x over previous
"""Trainium2 Bass kernel for nn_DecoderLSTMCell.

Computes, for B=16384 rows:
    gates = y @ W.T + h0 @ U.T + ctx @ C.T + b            # [B, 4H]
    i, f, o, g = split(gates, 4); i,f,o = sigmoid; g = tanh
    c = i * g + f * c0 ; h = o * tanh(c)
Returns (c, h), both [B, H] float32.

Strategy: data-parallel over the batch dim across 8 NeuronCores (2048
rows/core), weights replicated.  The host packs x = [y|h0|ctx] and
Wcat = [W|U|C] into transposed, partition-major bf16 layouts (the GEMM
runs in bf16 with fp32 PSUM accumulation either way; packing on the host
keeps the cores on the tensor-engine roofline).  Each core streams the
packed operands, runs its [2048 x 4096 x 4096] GEMM slice, and applies
the LSTM epilogue on the DVE/ACT engines directly from PSUM.

Per-core loop: batch passes of (512, 768, 768) rows x 8 hidden blocks
(e) x row-tile pairs; each (e, m) accumulates 32 matmuls of
[K=128]x[M=128]x[N=512] into one PSUM bank holding [i|f|o|g] x 128
hidden units for 128 batch rows.  The uneven pass sizes keep the first
e-block's DMA demand (x for the pass + first w tile = 8MB) under its
compute time so the PE ramp is stall-free; the fine first k-splits let
the first matmul issue ~1.5us after the first DMA.
Measured: ~916 us NEFF exec per core for the uniform-2-pass layout
(dense matmul floor ~874 us), max rel err vs fp32 reference ~8e-3.
"""

import ml_dtypes
import numpy as np

import concourse.tile as tile
import concourse.mybir as mybir
from concourse import bacc, bass_utils

P = 128
F32 = mybir.dt.float32
BF16 = mybir.dt.bfloat16
AF = mybir.ActivationFunctionType

# Problem shapes (hardcoded; see module docstring)
B, IN, H, CTX = 16384, 1024, 1024, 2048
KD = IN + H + CTX  # 4096 contraction dim
G = 4 * H
NCORES = 8
BC = B // NCORES  # 2048 batch rows per core
PASS_ROWS = (768, 768, 512)  # uneven: small-ish first pass for a stall-free ramp
WARMUP_MM = 0  # dummy matmuls on memset data to lift the PE clock gate early
CW = 256  # batch column chunk width of the packed x^T layout

LAST_RESULT = None  # BassKernelResults of the most recent run (for test.py)
_NC_CACHE = None  # compiled Bass module, reused across kernel() calls


def _splits(kt, first):
    """Sub-DMA k-tile split sizes.  The critical first loads use a fine
    ladder so each k-tile lands just ahead of the (initially clock-gated)
    matmul stream; steady-state loads use 4 coarse splits."""
    if not first:
        return [max(1, kt // 4)] * min(4, kt)
    # Post-warm-clock the PE consumes ~1.28us per k-tile (6-matmul group)
    # while the rings deliver ~1.07us per k-tile — split growth must stay
    # gentle or each step-up stalls the stream until its sub-DMA lands.
    ladder = [1, 1, 2, 2, 2, 4, 4, 4, 4, 4, 4]
    out, q = [], 0
    for sz in ladder:
        if q >= kt:
            break
        sz = min(sz, kt - q)
        out.append(sz)
        q += sz
    while q < kt:
        out.append(min(8, kt - q))
        q += out[-1]
    return out


def build_nc(bc=BC, h=H, kd=KD, pass_rows=PASS_ROWS, cw=CW, wtb_bufs=2):
    """Build the per-core SPMD Bass module.

    NEFF inputs (host-packed layouts):
      xTh : [bc//cw, P, kd//P, cw] bf16, xTh[ch,p,kt,b] = x[ch*cw+b, kt*P+p]
      wTh : [h//P, P, kd//P, 4P] bf16, wTh[e,p,kt,j*P+u] = Wcat[j*h+e*P+u, kt*P+p]
      c0s : [bc, h] f32
      bb  : [P, 4h] f32, bias broadcast along partitions, grouped like wTh:
            bb[:, e*4P + j*P + u] = b[j*h + e*P + u]
    NEFF outputs: c_out, h_out [bc, h] f32.
    """
    assert sum(pass_rows) == bc
    E = h // P
    KT = kd // P
    NW = 4 * P  # psum width: [i|f|o|g] x 128 hidden cols
    n_pass = len(pass_rows)

    nc = bacc.Bacc("TRN2", target_bir_lowering=False)
    xTh = nc.dram_tensor("xTh", (bc // cw, P, KT, cw), BF16, kind="ExternalInput")
    wTh = nc.dram_tensor("wTh", (E, P, KT, NW), BF16, kind="ExternalInput")
    c0s = nc.dram_tensor("c0s", (bc, h), F32, kind="ExternalInput")
    bb = nc.dram_tensor("bb", (P, 4 * h), F32, kind="ExternalInput")
    c_out = nc.dram_tensor("c_out", (bc, h), F32, kind="ExternalOutput")
    h_out = nc.dram_tensor("h_out", (bc, h), F32, kind="ExternalOutput")

    with (
        tile.TileContext(nc) as tc,
        tc.tile_pool(name="xp", bufs=2) as xp,
        tc.tile_pool(name="wp", bufs=wtb_bufs) as wp,
        tc.tile_pool(name="bp", bufs=2) as bp,
        tc.tile_pool(name="cp", bufs=4) as cp,
        tc.tile_pool(name="gp", bufs=3) as gp,
        tc.tile_pool(name="sp", bufs=3) as sp,
        tc.tile_pool(name="pp", bufs=(7 if WARMUP_MM else 8), space="PSUM") as pp,
        tc.tile_pool(name="wpp", bufs=1, space="PSUM") as wpp,
    ):
        # PE warm-up: the HAM clock gate needs ~3.4us of sustained matmul
        # activity before the PE runs at 2.4 GHz.  While the first real
        # operands stream in from HBM, burn a few matmuls on a zeroed
        # SBUF tile so the real stream starts (and stays) at full clock.
        if WARMUP_MM:
            wu = gp.tile([P, NW], BF16, tag="wu", name="warm_tile")
            nc.gpsimd.memset(wu[:], 0)
            wu_ps = wpp.tile([P, NW], F32, tag="wups", name="warm_ps")
            for _ in range(WARMUP_MM):
                nc.tensor.matmul(wu_ps[:], wu[:, 0:P], wu[:], start=True, stop=True)

        row_base = 0
        for p_i, rows in enumerate(pass_rows):
            NCP = rows // cw  # x^T chunks this pass
            MT = rows // P  # m tiles this pass
            chunk0 = row_base // cw
            # x^T chunks for this pass.  Pass 0's e-0 block runs all four
            # m-tiles as one group, so both chunks are consumed from k=0;
            # their fine-ladder sub-DMAs are issued interleaved so each
            # k-tile of both chunks lands just ahead of the matmul stream.
            xtb = [
                xp.tile([P, KT, cw], BF16, tag=f"xtb{mc}", name=f"xtb_{p_i}_{mc}")
                for mc in range(NCP)
            ]
            if p_i == 0:
                offs = [0] * NCP
                for sz in _splits(KT, first=True):
                    for mc in range(NCP):
                        q = offs[mc]
                        nc.scalar.dma_start(
                            out=xtb[mc][:, q:q + sz], in_=xTh[chunk0 + mc, :, q:q + sz]
                        )
                        offs[mc] += sz
            else:
                for mc in range(NCP):
                    q = 0
                    for sz in _splits(KT, first=False):
                        nc.scalar.dma_start(
                            out=xtb[mc][:, q:q + sz], in_=xTh[chunk0 + mc, :, q:q + sz]
                        )
                        q += sz
            for e in range(E):
                wt = wp.tile([P, KT, NW], BF16, tag="wtb", name=f"wtb_{p_i}_{e}")
                q = 0
                for sz in _splits(KT, first=(p_i == 0 and e == 0)):
                    nc.sync.dma_start(out=wt[:, q:q + sz], in_=wTh[e, :, q:q + sz])
                    q += sz
                bias_t = bp.tile([P, NW], F32, tag="bias", name=f"bias_{p_i}_{e}")
                nc.sync.dma_start(out=bias_t[:], in_=bb[:, e * NW:(e + 1) * NW])
                last = p_i == n_pass - 1 and e == E - 1
                # Pass 0 / e 0: one group of all 6 m-tiles — 6 matmuls per
                # k-tile keeps the warm-clock DMA demand (~250 GB/s) under
                # what the rings deliver, so the ramp never starves the PE.
                pstep = 1 if last else (6 if p_i == 0 and e == 0 else 2)
                for mp in range(0, MT, pstep):
                  pair = []
                  for m in range(mp, min(mp + pstep, MT)):
                    c0_t = cp.tile([P, P], F32, tag="c0", name=f"c0_{p_i}_{e}_{m}")
                    ps = pp.tile([P, NW], F32, tag="ps", name=f"ps_{p_i}_{e}_{m}")
                    pair.append((m, c0_t, ps))
                  for m, c0_t, ps in pair:
                    row0 = row_base + m * P
                    nc.sync.dma_start(
                        out=c0_t[:], in_=c0s[row0:row0 + P, e * P:(e + 1) * P]
                    )
                  for k in range(KT):
                    for m, c0_t, ps in pair:
                        mc, lc = divmod(m * P, cw)
                        nc.tensor.matmul(
                            ps[:],
                            xtb[mc][:, k, lc:lc + P],
                            wt[:, k, :],
                            start=(k == 0),
                            stop=(k == KT - 1),
                        )
                  for m, c0_t, ps in pair:
                    row0 = row_base + m * P
                    fin = last and m == MT - 1
                    # On the very last tile, route output DMAs through the
                    # idle sync queue and keep them off the ACT/DVE critical
                    # path (compute everything, then store).
                    dma_eng = nc.sync if fin else nc.scalar
                    ga = gp.tile([P, NW], F32, tag="ga", name=f"ga_{p_i}_{e}_{m}")
                    act = gp.tile([P, NW], F32, tag="act", name=f"act_{p_i}_{e}_{m}")
                    if fin:
                        # Finer-grained chain on the exposed last tile: get
                        # the g-gate through DVE+ACT before the rest so the
                        # downstream muls start earlier.
                        nc.vector.tensor_add(
                            ga[:, 3 * P:4 * P], ps[:, 3 * P:4 * P], bias_t[:, 3 * P:4 * P]
                        )
                        nc.scalar.activation(act[:, 3 * P:4 * P], ga[:, 3 * P:4 * P], AF.Tanh)
                        nc.vector.tensor_add(ga[:, 0:3 * P], ps[:, 0:3 * P], bias_t[:, 0:3 * P])
                        nc.scalar.activation(act[:, 0:P], ga[:, 0:P], AF.Sigmoid)
                        nc.scalar.activation(act[:, P:3 * P], ga[:, P:3 * P], AF.Sigmoid)
                    else:
                        nc.vector.tensor_add(ga[:], ps[:], bias_t[:])
                        nc.scalar.activation(act[:, 3 * P:4 * P], ga[:, 3 * P:4 * P], AF.Tanh)
                        nc.scalar.activation(act[:, 0:3 * P], ga[:, 0:3 * P], AF.Sigmoid)
                    ct = sp.tile([P, P], F32, tag="ct", name=f"ct_{p_i}_{e}_{m}")
                    nc.vector.tensor_mul(ct[:], act[:, 0:P], act[:, 3 * P:4 * P])
                    fc = sp.tile([P, P], F32, tag="fc", name=f"fc_{p_i}_{e}_{m}")
                    nc.vector.tensor_mul(fc[:], act[:, P:2 * P], c0_t[:])
                    nc.vector.tensor_add(ct[:], ct[:], fc[:])
                    if not fin:
                        nc.scalar.dma_start(
                            out=c_out[row0:row0 + P, e * P:(e + 1) * P], in_=ct[:]
                        )
                    tct = sp.tile([P, P], F32, tag="tct", name=f"tct_{p_i}_{e}_{m}")
                    nc.scalar.activation(tct[:], ct[:], AF.Tanh)
                    ht = sp.tile([P, P], F32, tag="ht", name=f"ht_{p_i}_{e}_{m}")
                    nc.vector.tensor_mul(ht[:], act[:, 2 * P:3 * P], tct[:])
                    if fin:
                        nc.sync.dma_start(
                            out=c_out[row0:row0 + P, e * P:(e + 1) * P], in_=ct[:]
                        )
                    dma_eng.dma_start(
                        out=h_out[row0:row0 + P, e * P:(e + 1) * P], in_=ht[:]
                    )
            row_base += rows
    nc.compile()
    return nc


def pack_inputs(y, ctx, c0, h0, W, U, C, b, bc=BC, h=H, kd=KD, cw=CW):
    """Host-side layout packing (pure data movement, no arithmetic)."""
    b_total = y.shape[0]
    E = h // P
    KT = kd // P
    x_all = np.concatenate([y, h0, ctx], axis=1)  # [B, KD]; order matches Wcat
    xTh = np.ascontiguousarray(
        x_all.reshape(b_total // cw, cw, KT, P).transpose(0, 3, 2, 1)
    ).astype(ml_dtypes.bfloat16)
    Wcat = np.concatenate([W, U, C], axis=1)  # [G, KD]
    wTh = np.ascontiguousarray(
        Wcat.reshape(4, E, P, KT, P).transpose(1, 4, 3, 0, 2).reshape(E, P, KT, 4 * P)
    ).astype(ml_dtypes.bfloat16)
    br = b.reshape(4, E, P).transpose(1, 0, 2).reshape(4 * h)
    bb = np.ascontiguousarray(np.broadcast_to(br, (P, 4 * h)))
    return xTh, wTh, bb


def kernel(y, ctx, c0, h0, W, U, C, b):
    global LAST_RESULT
    y = np.ascontiguousarray(np.asarray(y, dtype=np.float32))
    ctx = np.ascontiguousarray(np.asarray(ctx, dtype=np.float32))
    c0 = np.ascontiguousarray(np.asarray(c0, dtype=np.float32))
    h0 = np.ascontiguousarray(np.asarray(h0, dtype=np.float32))
    W = np.ascontiguousarray(np.asarray(W, dtype=np.float32))
    U = np.ascontiguousarray(np.asarray(U, dtype=np.float32))
    C = np.ascontiguousarray(np.asarray(C, dtype=np.float32))
    b = np.ascontiguousarray(np.asarray(b, dtype=np.float32))

    xTh, wTh, bb = pack_inputs(y, ctx, c0, h0, W, U, C, b)

    global _NC_CACHE
    if _NC_CACHE is None:
        _NC_CACHE = build_nc()
    nc = _NC_CACHE
    cpb = BC // CW  # x^T chunks per core
    in_maps = []
    for c_i in range(NCORES):
        in_maps.append(
            {
                "xTh": xTh[c_i * cpb:(c_i + 1) * cpb],
                "wTh": wTh,
                "c0s": np.ascontiguousarray(c0[c_i * BC:(c_i + 1) * BC]),
                "bb": bb,
            }
        )
    res = bass_utils.run_bass_kernel_spmd(nc, in_maps, core_ids=list(range(NCORES)))
    LAST_RESULT = res
    c_full = np.concatenate([r["c_out"] for r in res.results], axis=0)
    h_full = np.concatenate([r["h_out"] for r in res.results], axis=0)
    return (c_full, h_full)


# revision 16
# speedup vs baseline: 1.0032x; 1.0014x over previous
"""Trainium2 Bass kernel for nn_DecoderLSTMCell.

Computes, for B=16384 rows:
    gates = y @ W.T + h0 @ U.T + ctx @ C.T + b            # [B, 4H]
    i, f, o, g = split(gates, 4); i,f,o = sigmoid; g = tanh
    c = i * g + f * c0 ; h = o * tanh(c)
Returns (c, h), both [B, H] float32.

Strategy: data-parallel over the batch dim across 8 NeuronCores (2048
rows/core), weights replicated.  The host packs x = [y|h0|ctx] and
Wcat = [W|U|C] into transposed, partition-major bf16 layouts (the GEMM
runs in bf16 with fp32 PSUM accumulation either way; packing on the host
keeps the cores on the tensor-engine roofline).  Each core streams the
packed operands, runs its [2048 x 4096 x 4096] GEMM slice, and applies
the LSTM epilogue on the DVE/ACT engines directly from PSUM.

Per-core loop: batch passes of (512, 768, 768) rows x 8 hidden blocks
(e) x row-tile pairs; each (e, m) accumulates 32 matmuls of
[K=128]x[M=128]x[N=512] into one PSUM bank holding [i|f|o|g] x 128
hidden units for 128 batch rows.  The uneven pass sizes keep the first
e-block's DMA demand (x for the pass + first w tile = 8MB) under its
compute time so the PE ramp is stall-free; the fine first k-splits let
the first matmul issue ~1.5us after the first DMA.
Measured: ~916 us NEFF exec per core for the uniform-2-pass layout
(dense matmul floor ~874 us), max rel err vs fp32 reference ~8e-3.
"""

import ml_dtypes
import numpy as np

import concourse.tile as tile
import concourse.mybir as mybir
from concourse import bacc, bass_utils

P = 128
F32 = mybir.dt.float32
BF16 = mybir.dt.bfloat16
AF = mybir.ActivationFunctionType

# Problem shapes (hardcoded; see module docstring)
B, IN, H, CTX = 16384, 1024, 1024, 2048
KD = IN + H + CTX  # 4096 contraction dim
G = 4 * H
NCORES = 8
BC = B // NCORES  # 2048 batch rows per core
PASS_ROWS = (768, 768, 512)  # uneven: small-ish first pass for a stall-free ramp
WARMUP_MM = 0  # dummy matmuls on memset data to lift the PE clock gate early
CW = 256  # batch column chunk width of the packed x^T layout

LAST_RESULT = None  # BassKernelResults of the most recent run (for test.py)
_NC_CACHE = None  # compiled Bass module, reused across kernel() calls


def _splits(kt, first):
    """Sub-DMA k-tile split sizes.  The critical first loads use a fine
    ladder so each k-tile lands just ahead of the (initially clock-gated)
    matmul stream; steady-state loads use 4 coarse splits."""
    if not first:
        return [max(1, kt // 4)] * min(4, kt)
    # Post-warm-clock the PE consumes ~1.28us per k-tile (6-matmul group)
    # while the rings deliver ~1.07us per k-tile — split growth must stay
    # gentle or each step-up stalls the stream until its sub-DMA lands.
    ladder = [1, 1, 2, 2, 2, 4, 4, 4, 4, 4, 4]
    out, q = [], 0
    for sz in ladder:
        if q >= kt:
            break
        sz = min(sz, kt - q)
        out.append(sz)
        q += sz
    while q < kt:
        out.append(min(8, kt - q))
        q += out[-1]
    return out


def build_nc(bc=BC, h=H, kd=KD, pass_rows=PASS_ROWS, cw=CW, wtb_bufs=2):
    """Build the per-core SPMD Bass module.

    NEFF inputs (host-packed layouts):
      xTh : [bc//cw, P, kd//P, cw] bf16, xTh[ch,p,kt,b] = x[ch*cw+b, kt*P+p]
      wTh : [h//P, P, kd//P, 4P] bf16, wTh[e,p,kt,j*P+u] = Wcat[j*h+e*P+u, kt*P+p]
      c0s : [bc, h] f32
      bb  : [P, 4h] f32, bias broadcast along partitions, grouped like wTh:
            bb[:, e*4P + j*P + u] = b[j*h + e*P + u]
    NEFF outputs: c_out, h_out [bc, h] f32.
    """
    assert sum(pass_rows) == bc
    E = h // P
    KT = kd // P
    NW = 4 * P  # psum width: [i|f|o|g] x 128 hidden cols
    n_pass = len(pass_rows)

    nc = bacc.Bacc("TRN2", target_bir_lowering=False)
    xTh = nc.dram_tensor("xTh", (bc // cw, P, KT, cw), BF16, kind="ExternalInput")
    wTh = nc.dram_tensor("wTh", (E, P, KT, NW), BF16, kind="ExternalInput")
    c0s = nc.dram_tensor("c0s", (bc, h), F32, kind="ExternalInput")
    bb = nc.dram_tensor("bb", (P, 4 * h), F32, kind="ExternalInput")
    c_out = nc.dram_tensor("c_out", (bc, h), F32, kind="ExternalOutput")
    h_out = nc.dram_tensor("h_out", (bc, h), F32, kind="ExternalOutput")

    with (
        tile.TileContext(nc) as tc,
        tc.tile_pool(name="xp", bufs=2) as xp,
        tc.tile_pool(name="wp", bufs=wtb_bufs) as wp,
        tc.tile_pool(name="bp", bufs=2) as bp,
        tc.tile_pool(name="cp", bufs=4) as cp,
        tc.tile_pool(name="gp", bufs=3) as gp,
        tc.tile_pool(name="sp", bufs=3) as sp,
        tc.tile_pool(name="pp", bufs=(7 if WARMUP_MM else 8), space="PSUM") as pp,
        tc.tile_pool(name="wpp", bufs=1, space="PSUM") as wpp,
    ):
        # PE warm-up: the HAM clock gate needs ~3.4us of sustained matmul
        # activity before the PE runs at 2.4 GHz.  While the first real
        # operands stream in from HBM, burn a few matmuls on a zeroed
        # SBUF tile so the real stream starts (and stays) at full clock.
        if WARMUP_MM:
            wu = gp.tile([P, NW], BF16, tag="wu", name="warm_tile")
            nc.gpsimd.memset(wu[:], 0)
            wu_ps = wpp.tile([P, NW], F32, tag="wups", name="warm_ps")
            for _ in range(WARMUP_MM):
                nc.tensor.matmul(wu_ps[:], wu[:, 0:P], wu[:], start=True, stop=True)

        row_base = 0
        for p_i, rows in enumerate(pass_rows):
            NCP = rows // cw  # x^T chunks this pass
            MT = rows // P  # m tiles this pass
            chunk0 = row_base // cw
            # x^T chunks for this pass.  Pass 0's e-0 block runs all four
            # m-tiles as one group, so both chunks are consumed from k=0;
            # their fine-ladder sub-DMAs are issued interleaved so each
            # k-tile of both chunks lands just ahead of the matmul stream.
            xtb = [
                xp.tile([P, KT, cw], BF16, tag=f"xtb{mc}", name=f"xtb_{p_i}_{mc}")
                for mc in range(NCP)
            ]
            wt0 = None
            if p_i == 0:
                # Ramp prologue: interleave the k-split ladders of the three
                # x chunks AND the first w tile, alternating streams across
                # the two HWDGE rings so each carries ~160KB per k-tile
                # (round-robin serves the rings about evenly; an unbalanced
                # ring is what stalls the e-0 matmul group).
                wt0 = wp.tile([P, KT, NW], BF16, tag="wtb", name="wtb_0_0")
                offs = [0] * (NCP + 1)
                for si, sz in enumerate(_splits(KT, first=True)):
                    even = si % 2 == 0
                    q = offs[0]
                    nc.scalar.dma_start(
                        out=xtb[0][:, q:q + sz], in_=xTh[chunk0, :, q:q + sz]
                    )
                    offs[0] += sz
                    q = offs[1]
                    nc.sync.dma_start(
                        out=xtb[1][:, q:q + sz], in_=xTh[chunk0 + 1, :, q:q + sz]
                    )
                    offs[1] += sz
                    q = offs[2]
                    (nc.scalar if even else nc.sync).dma_start(
                        out=xtb[2][:, q:q + sz], in_=xTh[chunk0 + 2, :, q:q + sz]
                    )
                    offs[2] += sz
                    q = offs[3]
                    (nc.sync if even else nc.scalar).dma_start(
                        out=wt0[:, q:q + sz], in_=wTh[0, :, q:q + sz]
                    )
                    offs[3] += sz
            else:
                for mc in range(NCP):
                    q = 0
                    for sz in _splits(KT, first=False):
                        nc.scalar.dma_start(
                            out=xtb[mc][:, q:q + sz], in_=xTh[chunk0 + mc, :, q:q + sz]
                        )
                        q += sz
            for e in range(E):
                if p_i == 0 and e == 0:
                    wt = wt0  # prologue already issued its DMA ladder
                else:
                    wt = wp.tile([P, KT, NW], BF16, tag="wtb", name=f"wtb_{p_i}_{e}")
                    q = 0
                    for sz in _splits(KT, first=False):
                        nc.sync.dma_start(out=wt[:, q:q + sz], in_=wTh[e, :, q:q + sz])
                        q += sz
                bias_t = bp.tile([P, NW], F32, tag="bias", name=f"bias_{p_i}_{e}")
                nc.sync.dma_start(out=bias_t[:], in_=bb[:, e * NW:(e + 1) * NW])
                last = p_i == n_pass - 1 and e == E - 1
                # Pass 0 / e 0: one group of all 6 m-tiles — 6 matmuls per
                # k-tile keeps the warm-clock DMA demand (~250 GB/s) under
                # what the rings deliver, so the ramp never starves the PE.
                pstep = 1 if last else (6 if p_i == 0 and e == 0 else 2)
                for mp in range(0, MT, pstep):
                  pair = []
                  for m in range(mp, min(mp + pstep, MT)):
                    c0_t = cp.tile([P, P], F32, tag="c0", name=f"c0_{p_i}_{e}_{m}")
                    ps = pp.tile([P, NW], F32, tag="ps", name=f"ps_{p_i}_{e}_{m}")
                    pair.append((m, c0_t, ps))
                  for m, c0_t, ps in pair:
                    row0 = row_base + m * P
                    nc.sync.dma_start(
                        out=c0_t[:], in_=c0s[row0:row0 + P, e * P:(e + 1) * P]
                    )
                  for k in range(KT):
                    for m, c0_t, ps in pair:
                        mc, lc = divmod(m * P, cw)
                        nc.tensor.matmul(
                            ps[:],
                            xtb[mc][:, k, lc:lc + P],
                            wt[:, k, :],
                            start=(k == 0),
                            stop=(k == KT - 1),
                        )
                  for m, c0_t, ps in pair:
                    row0 = row_base + m * P
                    fin = last and m == MT - 1
                    # On the very last tile, route output DMAs through the
                    # idle sync queue and keep them off the ACT/DVE critical
                    # path (compute everything, then store).
                    dma_eng = nc.sync if fin else nc.scalar
                    ga = gp.tile([P, NW], F32, tag="ga", name=f"ga_{p_i}_{e}_{m}")
                    act = gp.tile([P, NW], F32, tag="act", name=f"act_{p_i}_{e}_{m}")
                    if fin:
                        # Finer-grained chain on the exposed last tile: get
                        # the g-gate through DVE+ACT before the rest so the
                        # downstream muls start earlier.
                        nc.vector.tensor_add(
                            ga[:, 3 * P:4 * P], ps[:, 3 * P:4 * P], bias_t[:, 3 * P:4 * P]
                        )
                        nc.scalar.activation(act[:, 3 * P:4 * P], ga[:, 3 * P:4 * P], AF.Tanh)
                        nc.vector.tensor_add(ga[:, 0:3 * P], ps[:, 0:3 * P], bias_t[:, 0:3 * P])
                        nc.scalar.activation(act[:, 0:P], ga[:, 0:P], AF.Sigmoid)
                        nc.scalar.activation(act[:, P:3 * P], ga[:, P:3 * P], AF.Sigmoid)
                    else:
                        nc.vector.tensor_add(ga[:], ps[:], bias_t[:])
                        nc.scalar.activation(act[:, 3 * P:4 * P], ga[:, 3 * P:4 * P], AF.Tanh)
                        nc.scalar.activation(act[:, 0:3 * P], ga[:, 0:3 * P], AF.Sigmoid)
                    ct = sp.tile([P, P], F32, tag="ct", name=f"ct_{p_i}_{e}_{m}")
                    nc.vector.tensor_mul(ct[:], act[:, 0:P], act[:, 3 * P:4 * P])
                    fc = sp.tile([P, P], F32, tag="fc", name=f"fc_{p_i}_{e}_{m}")
                    nc.vector.tensor_mul(fc[:], act[:, P:2 * P], c0_t[:])
                    nc.vector.tensor_add(ct[:], ct[:], fc[:])
                    if not fin:
                        nc.scalar.dma_start(
                            out=c_out[row0:row0 + P, e * P:(e + 1) * P], in_=ct[:]
                        )
                    tct = sp.tile([P, P], F32, tag="tct", name=f"tct_{p_i}_{e}_{m}")
                    nc.scalar.activation(tct[:], ct[:], AF.Tanh)
                    ht = sp.tile([P, P], F32, tag="ht", name=f"ht_{p_i}_{e}_{m}")
                    nc.vector.tensor_mul(ht[:], act[:, 2 * P:3 * P], tct[:])
                    if fin:
                        nc.sync.dma_start(
                            out=c_out[row0:row0 + P, e * P:(e + 1) * P], in_=ct[:]
                        )
                    dma_eng.dma_start(
                        out=h_out[row0:row0 + P, e * P:(e + 1) * P], in_=ht[:]
                    )
            row_base += rows
    nc.compile()
    return nc


def pack_inputs(y, ctx, c0, h0, W, U, C, b, bc=BC, h=H, kd=KD, cw=CW):
    """Host-side layout packing (pure data movement, no arithmetic)."""
    b_total = y.shape[0]
    E = h // P
    KT = kd // P
    x_all = np.concatenate([y, h0, ctx], axis=1)  # [B, KD]; order matches Wcat
    xTh = np.ascontiguousarray(
        x_all.reshape(b_total // cw, cw, KT, P).transpose(0, 3, 2, 1)
    ).astype(ml_dtypes.bfloat16)
    Wcat = np.concatenate([W, U, C], axis=1)  # [G, KD]
    wTh = np.ascontiguousarray(
        Wcat.reshape(4, E, P, KT, P).transpose(1, 4, 3, 0, 2).reshape(E, P, KT, 4 * P)
    ).astype(ml_dtypes.bfloat16)
    br = b.reshape(4, E, P).transpose(1, 0, 2).reshape(4 * h)
    bb = np.ascontiguousarray(np.broadcast_to(br, (P, 4 * h)))
    return xTh, wTh, bb


def kernel(y, ctx, c0, h0, W, U, C, b):
    global LAST_RESULT
    y = np.ascontiguousarray(np.asarray(y, dtype=np.float32))
    ctx = np.ascontiguousarray(np.asarray(ctx, dtype=np.float32))
    c0 = np.ascontiguousarray(np.asarray(c0, dtype=np.float32))
    h0 = np.ascontiguousarray(np.asarray(h0, dtype=np.float32))
    W = np.ascontiguousarray(np.asarray(W, dtype=np.float32))
    U = np.ascontiguousarray(np.asarray(U, dtype=np.float32))
    C = np.ascontiguousarray(np.asarray(C, dtype=np.float32))
    b = np.ascontiguousarray(np.asarray(b, dtype=np.float32))

    xTh, wTh, bb = pack_inputs(y, ctx, c0, h0, W, U, C, b)

    global _NC_CACHE
    if _NC_CACHE is None:
        _NC_CACHE = build_nc()
    nc = _NC_CACHE
    cpb = BC // CW  # x^T chunks per core
    in_maps = []
    for c_i in range(NCORES):
        in_maps.append(
            {
                "xTh": xTh[c_i * cpb:(c_i + 1) * cpb],
                "wTh": wTh,
                "c0s": np.ascontiguousarray(c0[c_i * BC:(c_i + 1) * BC]),
                "bb": bb,
            }
        )
    res = bass_utils.run_bass_kernel_spmd(nc, in_maps, core_ids=list(range(NCORES)))
    LAST_RESULT = res
    c_full = np.concatenate([r["c_out"] for r in res.results], axis=0)
    h_full = np.concatenate([r["h_out"] for r in res.results], axis=0)
    return (c_full, h_full)
